# revision 52
# baseline (speedup 1.0000x reference)
"""MoE top-1 routing kernel for Trainium2 (8 NeuronCores, expert-parallel).

Strategy:
  - Gate (x @ Wg.T + bg, argmax) is computed on host in float64. The min
    top-2 logit gap for this problem's data is ~1.2e-5, orders of magnitude
    above any fp32 backend's rounding noise (~1e-6), so the fp64 argmax
    matches the fp32 reference argmax exactly.
  - Tokens are grouped by expert on host (the "all-to-all dispatch");
    core e receives expert e's tokens (capacity-padded) plus expert e's
    weights, and runs the dense SwiGLU FFN for just those tokens.
  - Outputs are scattered back to token order on host (the "combine").
    With top-1 routing the combine weight is exactly 1.0.

Device kernel (per core): fp8(e4m3) DoubleRow matmuls with first-order
residual correction. Every operand T is split on host into
T_hi = fp8(T*S) and T_lo = fp8(T*S - T_hi) at the same scale S, and each
matmul stage computes the three bilinear terms
  W_hi*a_hi + W_lo*a_hi + W_hi*a_lo
accumulated in one PSUM group (all terms share one product scale because
hi/lo use the same scale). The dropped W_lo*a_lo term is second order
(~0.2% end-to-end vs the fp32 reference, measured). DoubleRow packs two
128-row k-tiles per matmul (256-deep contraction) at 0.5 cycles/column,
so the 3-term scheme costs 0.75x the bf16 schedule.

Per chunk of nn token-columns:
  stage A: ps1[m] = sum_terms W1.T x   (9 matmuls per f-tile m)
           ps2[m] = sum_terms W2.T x
           sil = silu(c1*ps1)                      (ACT)
           gsf = (sil*cg)*ps2                      (DVE, fp32)
           g_hi = fp8(gsf)                         (ACT)
           g_lo = fp8(gsf - g_hi)                  (DVE)
  stage B: pso[d] = sum_terms W3.T g   (6 matmuls per d-tile)
           y[d] = cy*pso[d] -> bf16                (ACT/DVE alternating)
Host applies no further scaling: cy folds all dequant factors.

Scheduling notes (from TimelineSim traces):
  - software pipeline A(i+1) before B(i): the PE stream never waits on a
    chunk's silu/quant chain;
  - each (chunk, d-pair) writes its own DRAM tensor: the Tile DRAM dep
    tracker is whole-tensor, a shared output would serialize stores;
  - stores ride three queues (ACT/Pool/SP); the last chunk does one merged
    store so only a single HWDGE slot sits in the exposed tail;
  - all x loads are issued upfront on SP so no load ever queues behind a
    store's semaphore wait.
"""

import sys
from contextlib import ExitStack

if "/opt/trn_rl_repo" not in sys.path:
    sys.path.insert(0, "/opt/trn_rl_repo")

import numpy as np

P = 128
D = 768          # model dim
E = 8            # experts == cores
F = 469          # ffn hidden
FP = 512         # F padded to a multiple of 128
DK = 3           # double-k tiles over D (6x128 = 3x256)
FK = 2           # double-k tiles over FP (4x128 = 2x256)
MT = 4           # f-tiles (FP/128)
DT = 6           # d-tiles (D/128)
MIN_C = 1024     # capacity floor (also keeps the multi-chunk pipeline shape)
CHUNK = 512      # max chunk (one PSUM bank of fp32)

# power-of-two quantization scales (host): exact in fp32
SX = 16.0        # |x| max ~5.5  -> 88  (< 120 safety vs e4m3 max 240)
SW = 1024.0      # |W| max ~0.11 -> 115
SG = 16.0        # |g| max ~7    -> 112
C1 = 1.0 / (SX * SW)    # dequant for silu input
CG = SG / (SX * SW)     # gsf = (sil*CG)*ps2 == g*SG
CY = 1.0 / (SG * SW)    # final dequant

_cache = {}


def _chunks_of(C):
    """Remainder chunk SECOND-TO-LAST: its small stage-B lands early (covered
    by the final wide chunk's matmuls), while chunk 0 stays wide so the PE has
    work during the weight preload. The final chunk is wide so its own store
    chain is the only exposed tail."""
    sizes = []
    left = C
    while left > CHUNK:
        sizes.append(CHUNK)
        left -= CHUNK
    return sizes + [left]


def _build(C):
    import concourse.bacc as bacc
    import concourse.tile as tile
    from concourse import mybir

    f32 = mybir.dt.float32
    bf16 = mybir.dt.bfloat16
    e4 = mybir.dt.float8e4
    DR = mybir.MatmulPerfMode.DoubleRow
    silu = mybir.ActivationFunctionType.Silu
    copyf = mybir.ActivationFunctionType.Copy

    assert C % 2 == 0, C
    chunks = []
    off = 0
    for nn in _chunks_of(C):
        chunks.append((off, nn))
        off += nn
    assert off == C

    nc = bacc.Bacc("TRN2", target_bir_lowering=False, debug=False, num_devices=E)

    xh = nc.dram_tensor("xh", [2 * DK, P, C], e4, kind="ExternalInput").ap()
    xl = nc.dram_tensor("xl", [2 * DK, P, C], e4, kind="ExternalInput").ap()
    # W1/W2 come as f-halves in p-major (= SBUF) layout: each half is one
    # fully-contiguous full-bandwidth DMA, so the preload streams in the
    # exact order the chunk-0 m-tile pairs consume it
    wab = {}
    for wn in ("w1h", "w1l", "w2h", "w2l"):
        for half in "ab":
            wab[wn, half] = nc.dram_tensor(
                f"{wn}{half}", [P, 2 * DK, FP // 2], e4, kind="ExternalInput"
            ).ap()
    w3h = nc.dram_tensor("w3h", [2 * FK, P, D], e4, kind="ExternalInput").ap()
    w3l = nc.dram_tensor("w3l", [2 * FK, P, D], e4, kind="ExternalInput").ap()
    # one output tensor per (chunk, d-tile-pair): the Tile DRAM dependency
    # tracker is whole-tensor, so sharing one yt would serialize stores on
    # false WAW edges; dedicated tensors let every store fire the moment its
    # two copies land
    youts = {}
    sizes = _chunks_of(C)
    for ci, nn in enumerate(sizes):
        if ci == len(sizes) - 1:
            youts[ci, 0] = nc.dram_tensor(
                f"y{ci}_0", [P, DT, nn], bf16, kind="ExternalOutput"
            ).ap()
        else:
            for j in range(DT // 2):
                youts[ci, j] = nc.dram_tensor(
                    f"y{ci}_{j}", [P, 2, nn], bf16, kind="ExternalOutput"
                ).ap()

    with tile.TileContext(nc) as tc, ExitStack() as ctx:
        wpool = ctx.enter_context(tc.tile_pool(name="w", bufs=1))
        xpool = ctx.enter_context(tc.tile_pool(name="x", bufs=3))
        gpool = ctx.enter_context(tc.tile_pool(name="g", bufs=2))
        spool = ctx.enter_context(tc.tile_pool(name="s", bufs=4))
        opool = ctx.enter_context(tc.tile_pool(name="o", bufs=6))
        pspool = ctx.enter_context(tc.tile_pool(name="ps", bufs=8, space="PSUM"))

        # SBUF weight tiles: [P, 2*DK, FP] so [:, 2dk:2dk+2, mslice] is a
        # [128, 2, 128] DoubleRow stationary operand (256-deep contraction)
        wsb = {}
        for wn in ("w1h", "w1l", "w2h", "w2l"):
            for half in "ab":
                wsb[wn, half] = wpool.tile(
                    [P, 2 * DK, FP // 2], e4, tag=f"{wn}{half}", name=f"{wn}{half}_sb"
                )
        w1h_sb = (wsb["w1h", "a"], wsb["w1h", "b"])
        w1l_sb = (wsb["w1l", "a"], wsb["w1l", "b"])
        w2h_sb = (wsb["w2h", "a"], wsb["w2h", "b"])
        w2l_sb = (wsb["w2l", "a"], wsb["w2l", "b"])
        w3h_sb = wpool.tile([P, 2 * FK, D], e4, tag="w3h", name="w3h_sb")
        w3l_sb = wpool.tile([P, 2 * FK, D], e4, tag="w3l", name="w3l_sb")

        # ---- DMA preload; first-matmul deps (w1h dk0 + x0h dk0) land first
        nn0 = chunks[0][1]
        x0h = xpool.tile([P, 2 * DK, nn0], e4, tag="xh", name="x0h")
        x0l = xpool.tile([P, 2 * DK, nn0], e4, tag="xl", name="x0l")
        loads = [
            (w1h_sb[:, 0:2], w1h[0:2].rearrange("k p f -> p k f")),
            (x0h[:, 0:2], xh[0:2, :, 0:nn0].rearrange("k p n -> p k n")),
            (w1h_sb[:, 2:4], w1h[2:4].rearrange("k p f -> p k f")),
            (x0h[:, 2:4], xh[2:4, :, 0:nn0].rearrange("k p n -> p k n")),
            (w1h_sb[:, 4:], w1h[4:].rearrange("k p f -> p k f")),
            (x0h[:, 4:], xh[4:, :, 0:nn0].rearrange("k p n -> p k n")),
            (x0l[:], xl[:, :, 0:nn0].rearrange("k p n -> p k n")),
            (w1l_sb[:], w1l.rearrange("k p f -> p k f")),
            (w2h_sb[:], w2h.rearrange("k p f -> p k f")),
            (w2l_sb[:], w2l.rearrange("k p f -> p k f")),
        ]
        wloads = [
            (w3h_sb[:], w3h.rearrange("k p d -> p k d")),
            (w3l_sb[:], w3l.rearrange("k p d -> p k d")),
        ]
        xtiles = {0: (x0h, x0l)}
        for ci, (n0, nn) in enumerate(chunks):
            if ci == 0:
                continue
            xnh = xpool.tile([P, 2 * DK, nn], e4, tag="xh", name=f"x{ci}h")
            xnl = xpool.tile([P, 2 * DK, nn], e4, tag="xl", name=f"x{ci}l")
            loads.append((xnh[:], xh[:, :, n0 : n0 + nn].rearrange("k p n -> p k n")))
            loads.append((xnl[:], xl[:, :, n0 : n0 + nn].rearrange("k p n -> p k n")))
            if ci == 1:
                loads.extend(wloads)  # w3 lands after x1 (needed later)
            xtiles[ci] = (xnh, xnl)
        # alternate the two HWDGE-capable queues (SP, ACT): the per-DMA
        # ~650ns SEQ+HWDGE issue cost would otherwise serialize the preload
        engines = (nc.sync, nc.scalar)
        for li, (dst, srcap) in enumerate(loads):
            engines[li % 2].dma_start(dst, srcap)

        def ms_slice(m):
            return slice(m * P, (m + 1) * P)

        gtiles = {}

        def _mm_group(ps, terms, ms):
            for ti, (w, xn) in enumerate(terms):
                for dk in range(DK):
                    for m in ms:
                        nc.tensor.matmul(
                            ps[m][:],
                            w[m // 2][:, 2 * dk : 2 * dk + 2, ms_slice(m % 2)],
                            xn[:, 2 * dk : 2 * dk + 2],
                            start=(ti == 0 and dk == 0),
                            stop=(ti == 2 and dk == DK - 1),
                            perf_mode=DR,
                        )

        def _chain(m, ps1, ps2, g_hi, g_lo, on_pool=False):
            sil = spool.tile([P, nn_of[m]], f32, tag="sil", name="sil")
            nc.scalar.activation(sil[:], ps1[m][:], silu, scale=C1)
            gsf = spool.tile([P, nn_of[m]], f32, tag="gsf", name="gsf")
            # gsf = (sil * CG) * ps2  == g * SG
            nc.vector.scalar_tensor_tensor(
                gsf[:], sil[:], CG, ps2[m][:],
                mybir.AluOpType.mult, mybir.AluOpType.mult,
            )
            if on_pool:
                # last chunk: quantize on Pool so ACT/DVE stay free for the
                # previous chunk's output copies
                nc.gpsimd.tensor_copy(g_hi[:, m], gsf[:])
                nc.gpsimd.tensor_sub(g_lo[:, m], gsf[:], g_hi[:, m])
            else:
                nc.scalar.activation(g_hi[:, m], gsf[:], copyf)
                nc.vector.tensor_sub(g_lo[:, m], gsf[:], g_hi[:, m])

        nn_of = {}

        def stage_a(ci):
            n0, nn = chunks[ci]
            xnh, xnl = xtiles[ci]
            for m in range(MT):
                nn_of[m] = nn
            # g tiles: [:, 2fk:2fk+2] is the [128, 2, nn] stage-B moving operand
            g_hi = gpool.tile([P, 2 * FK, nn], e4, tag="ghi", name=f"ghi{ci}")
            g_lo = gpool.tile([P, 2 * FK, nn], e4, tag="glo", name=f"glo{ci}")
            gtiles[ci] = (g_hi, g_lo)
            t1 = [(w1h_sb, xnh), (w1h_sb, xnl), (w1l_sb, xnh)]
            t2 = [(w2h_sb, xnh), (w2h_sb, xnl), (w2l_sb, xnh)]
            for ms in ((0, 1), (2, 3)):
                ps1 = {
                    m: pspool.tile([P, nn], f32, tag="ps", name=f"ps1_{m}")
                    for m in ms
                }
                _mm_group(ps1, t1, ms)
                ps2 = {
                    m: pspool.tile([P, nn], f32, tag="ps", name=f"ps2_{m}")
                    for m in ms
                }
                _mm_group(ps2, t2, ms)
                for m in ms:
                    _chain(m, ps1, ps2, g_hi, g_lo)

        def stage_b(ci, is_last):
            n0, nn = chunks[ci]
            g_hi, g_lo = gtiles[ci]
            ot = None
            otfull = None
            if is_last:
                otfull = opool.tile([P, DT, nn], bf16, tag="otf", name="otfull")
            store_eng = (nc.scalar, nc.gpsimd, nc.sync)
            copy_eng = (nc.vector, nc.scalar) * 3
            for d in range(DT):
                pso = pspool.tile([P, nn], f32, tag="ps", name="pso")
                first, last = (0, 0), (1, 2)
                for fk in range(FK):
                    for ti, (w, g) in enumerate(
                        [(w3h_sb, g_hi), (w3l_sb, g_hi), (w3h_sb, g_lo)]
                    ):
                        nc.tensor.matmul(
                            pso[:],
                            w[:, 2 * fk : 2 * fk + 2, d * P : (d + 1) * P],
                            g[:, 2 * fk : 2 * fk + 2],
                            start=((fk, ti) == first),
                            stop=((fk, ti) == last),
                            perf_mode=DR,
                        )
                if is_last:
                    dst = otfull[:, d]
                else:
                    if d % 2 == 0:
                        ot = opool.tile(
                            [P, 2, nn], bf16, tag="ot", name=f"ot{d // 2}"
                        )
                    dst = ot[:, d % 2]
                ce = copy_eng[d]
                if ce is nc.scalar:
                    ce.activation(dst, pso[:], copyf, scale=CY)
                else:
                    ce.tensor_scalar_mul(dst, pso[:], CY)
                if not is_last and d % 2 == 1:
                    store_eng[d // 2].dma_start(youts[ci, d // 2][:], ot[:])
            if is_last:
                # one merged store: a single HWDGE slot in the tail window
                nc.sync.dma_start(youts[ci, 0][:], otfull[:])


        # software pipeline: A(i+1) is emitted before B(i) so the PE stream
        # never waits on chunk i's silu/quant chain
        nchunks = len(chunks)
        stage_a(0)
        for ci in range(1, nchunks):
            stage_a(ci)
            stage_b(ci - 1, False)
        stage_b(nchunks - 1, True)

    nc.compile()
    return nc


LAST_RESULTS = None  # BassKernelResults of the most recent run (for test harness)


def _e4(a):
    import ml_dtypes

    return np.ascontiguousarray(a).astype(ml_dtypes.float8_e4m3)


def _split_hi_lo(a32):
    """fp8 split: a32 (already scaled, fp32) -> (hi, lo) e4m3 at one scale."""
    hi = _e4(a32)
    lo = _e4(a32 - hi.astype(np.float32))
    return hi, lo


def _pack_dr(mat, kt):
    """[K, N] -> [2*kt, P, N] plain k-tile layout (DoubleRow pairs adjacent)."""
    K, N = mat.shape
    assert K == kt * 2 * P
    return np.ascontiguousarray(mat.reshape(2 * kt, P, N))


def kernel(x, Wg, bg, W1, W2, W3):
    global LAST_RESULTS
    from concourse.bass_utils import run_bass_kernel_spmd

    x = np.asarray(x)
    Wg, bg = np.asarray(Wg), np.asarray(bg)
    W1, W2, W3 = np.asarray(W1), np.asarray(W2), np.asarray(W3)
    B, S, d = x.shape
    T = B * S
    assert d == D and Wg.shape == (E, D)

    xf = np.ascontiguousarray(x.reshape(T, D))

    # ---- host gate + top-1 routing (fp64: exact vs any fp32 backend) ----
    gate = xf.astype(np.float64) @ Wg.astype(np.float64).T + bg.astype(np.float64)
    eid = np.argmax(gate, axis=1)
    counts = np.bincount(eid, minlength=E)
    order = np.argsort(eid, kind="stable")
    offs = np.concatenate(([0], np.cumsum(counts)))

    C = max(MIN_C, 2 * int(-(-counts.max() // 2)))
    if C not in _cache:
        _cache[C] = _build(C)
    nc = _cache[C]

    # ---- build per-core inputs (dispatch) ----
    in_maps = []
    tok_lists = []
    for e in range(E):
        toks = order[offs[e] : offs[e + 1]]
        tok_lists.append(toks)
        ce = len(toks)
        xeT = np.zeros((D, C), dtype=np.float32)
        if ce:
            xeT[:, :ce] = xf[toks].T
        xh_, xl_ = _split_hi_lo(xeT * SX)

        w1 = np.zeros((D, FP), dtype=np.float32)
        w1[:, :F] = W1[e].T
        w2 = np.zeros((D, FP), dtype=np.float32)
        w2[:, :F] = W2[e].T
        w3 = np.zeros((FP, D), dtype=np.float32)
        w3[:F, :] = W3[e].T
        w1h_, w1l_ = _split_hi_lo(w1 * SW)
        w2h_, w2l_ = _split_hi_lo(w2 * SW)
        w3h_, w3l_ = _split_hi_lo(w3 * SW)

        im = {
            "xh": _pack_dr(xh_, DK),
            "xl": _pack_dr(xl_, DK),
            "w3h": _pack_dr(w3h_, FK),
            "w3l": _pack_dr(w3l_, FK),
        }
        for wn, wm in (("w1h", w1h_), ("w1l", w1l_), ("w2h", w2h_), ("w2l", w2l_)):
            pm = _pack_dr(wm, DK).transpose(1, 0, 2)  # [P, 2DK, FP] p-major
            im[wn + "a"] = np.ascontiguousarray(pm[:, :, : FP // 2])
            im[wn + "b"] = np.ascontiguousarray(pm[:, :, FP // 2 :])
        in_maps.append(im)

    res = run_bass_kernel_spmd(nc, in_maps, list(range(E)))
    LAST_RESULTS = res

    # ---- combine: scatter outputs back to token order ----
    y = np.empty((T, D), dtype=np.float32)
    for e in range(E):
        toks = tok_lists[e]
        if len(toks):
            r = res.results[e]
            sizes = _chunks_of(C)
            cols = []
            for ci, nn in enumerate(sizes):
                if ci == len(sizes) - 1:
                    blk = r[f"y{ci}_0"].reshape(P, DT, nn).transpose(1, 0, 2)
                else:
                    pairs = [
                        r[f"y{ci}_{j}"].reshape(P, 2, nn)
                        for j in range(DT // 2)
                    ]
                    blk = (
                        np.stack(pairs, 0).transpose(0, 2, 1, 3)
                    )  # [3, 2, P, nn]
                cols.append(blk.reshape(D, nn))
            yte = np.concatenate(cols, axis=1).astype(np.float32)
            assert np.isfinite(yte).all(), "fp8 overflow on core %d" % e
            y[toks] = yte[:, : len(toks)].T
    return y.reshape(B, S, d)


# revision 53
# speedup vs baseline: 1.0023x; 1.0023x over previous
"""MoE top-1 routing kernel for Trainium2 (8 NeuronCores, expert-parallel).

Strategy:
  - Gate (x @ Wg.T + bg, argmax) is computed on host in float64. The min
    top-2 logit gap for this problem's data is ~1.2e-5, orders of magnitude
    above any fp32 backend's rounding noise (~1e-6), so the fp64 argmax
    matches the fp32 reference argmax exactly.
  - Tokens are grouped by expert on host (the "all-to-all dispatch");
    core e receives expert e's tokens (capacity-padded) plus expert e's
    weights, and runs the dense SwiGLU FFN for just those tokens.
  - Outputs are scattered back to token order on host (the "combine").
    With top-1 routing the combine weight is exactly 1.0.

Device kernel (per core): fp8(e4m3) DoubleRow matmuls with first-order
residual correction. Every operand T is split on host into
T_hi = fp8(T*S) and T_lo = fp8(T*S - T_hi) at the same scale S, and each
matmul stage computes the three bilinear terms
  W_hi*a_hi + W_lo*a_hi + W_hi*a_lo
accumulated in one PSUM group (all terms share one product scale because
hi/lo use the same scale). The dropped W_lo*a_lo term is second order
(~0.2% end-to-end vs the fp32 reference, measured). DoubleRow packs two
128-row k-tiles per matmul (256-deep contraction) at 0.5 cycles/column,
so the 3-term scheme costs 0.75x the bf16 schedule.

Per chunk of nn token-columns:
  stage A: ps1[m] = sum_terms W1.T x   (9 matmuls per f-tile m)
           ps2[m] = sum_terms W2.T x
           sil = silu(c1*ps1)                      (ACT)
           gsf = (sil*cg)*ps2                      (DVE, fp32)
           g_hi = fp8(gsf)                         (ACT)
           g_lo = fp8(gsf - g_hi)                  (DVE)
  stage B: pso[d] = sum_terms W3.T g   (6 matmuls per d-tile)
           y[d] = cy*pso[d] -> bf16                (ACT/DVE alternating)
Host applies no further scaling: cy folds all dequant factors.

Scheduling notes (from TimelineSim traces):
  - software pipeline A(i+1) before B(i): the PE stream never waits on a
    chunk's silu/quant chain;
  - each (chunk, d-pair) writes its own DRAM tensor: the Tile DRAM dep
    tracker is whole-tensor, a shared output would serialize stores;
  - stores ride three queues (ACT/Pool/SP); the last chunk does one merged
    store so only a single HWDGE slot sits in the exposed tail;
  - all x loads are issued upfront on SP so no load ever queues behind a
    store's semaphore wait.
"""

import sys
from contextlib import ExitStack

if "/opt/trn_rl_repo" not in sys.path:
    sys.path.insert(0, "/opt/trn_rl_repo")

import numpy as np

P = 128
D = 768          # model dim
E = 8            # experts == cores
F = 469          # ffn hidden
FP = 512         # F padded to a multiple of 128
DK = 3           # double-k tiles over D (6x128 = 3x256)
FK = 2           # double-k tiles over FP (4x128 = 2x256)
MT = 4           # f-tiles (FP/128)
DT = 6           # d-tiles (D/128)
MIN_C = 1024     # capacity floor (also keeps the multi-chunk pipeline shape)
CHUNK = 512      # max chunk (one PSUM bank of fp32)

# power-of-two quantization scales (host): exact in fp32
SX = 16.0        # |x| max ~5.5  -> 88  (< 120 safety vs e4m3 max 240)
SW = 1024.0      # |W| max ~0.11 -> 115
SG = 16.0        # |g| max ~7    -> 112
C1 = 1.0 / (SX * SW)    # dequant for silu input
CG = SG / (SX * SW)     # gsf = (sil*CG)*ps2 == g*SG
CY = 1.0 / (SG * SW)    # final dequant

_cache = {}


def _chunks_of(C):
    """Remainder chunk SECOND-TO-LAST: its small stage-B lands early (covered
    by the final wide chunk's matmuls), while chunk 0 stays wide so the PE has
    work during the weight preload. The final chunk is wide so its own store
    chain is the only exposed tail."""
    sizes = []
    left = C
    while left > CHUNK:
        sizes.append(CHUNK)
        left -= CHUNK
    return sizes + [left]


def _build(C):
    import concourse.bacc as bacc
    import concourse.tile as tile
    from concourse import mybir

    f32 = mybir.dt.float32
    bf16 = mybir.dt.bfloat16
    e4 = mybir.dt.float8e4
    DR = mybir.MatmulPerfMode.DoubleRow
    silu = mybir.ActivationFunctionType.Silu
    copyf = mybir.ActivationFunctionType.Copy

    assert C % 2 == 0, C
    chunks = []
    off = 0
    for nn in _chunks_of(C):
        chunks.append((off, nn))
        off += nn
    assert off == C

    nc = bacc.Bacc("TRN2", target_bir_lowering=False, debug=False, num_devices=E)

    xh = nc.dram_tensor("xh", [2 * DK, P, C], e4, kind="ExternalInput").ap()
    xl = nc.dram_tensor("xl", [2 * DK, P, C], e4, kind="ExternalInput").ap()
    w1h = nc.dram_tensor("w1h", [2 * DK, P, FP], e4, kind="ExternalInput").ap()
    w1l = nc.dram_tensor("w1l", [2 * DK, P, FP], e4, kind="ExternalInput").ap()
    w2h = nc.dram_tensor("w2h", [2 * DK, P, FP], e4, kind="ExternalInput").ap()
    w2l = nc.dram_tensor("w2l", [2 * DK, P, FP], e4, kind="ExternalInput").ap()
    w3h = nc.dram_tensor("w3h", [2 * FK, P, D], e4, kind="ExternalInput").ap()
    w3l = nc.dram_tensor("w3l", [2 * FK, P, D], e4, kind="ExternalInput").ap()
    # one output tensor per (chunk, d-tile-pair): the Tile DRAM dependency
    # tracker is whole-tensor, so sharing one yt would serialize stores on
    # false WAW edges; dedicated tensors let every store fire the moment its
    # two copies land
    youts = {}
    sizes = _chunks_of(C)
    for ci, nn in enumerate(sizes):
        if ci == len(sizes) - 1:
            youts[ci, 0] = nc.dram_tensor(
                f"y{ci}_0", [P, DT, nn], bf16, kind="ExternalOutput"
            ).ap()
        else:
            for j in range(DT // 2):
                youts[ci, j] = nc.dram_tensor(
                    f"y{ci}_{j}", [P, 2, nn], bf16, kind="ExternalOutput"
                ).ap()

    with tile.TileContext(nc) as tc, ExitStack() as ctx:
        wpool = ctx.enter_context(tc.tile_pool(name="w", bufs=1))
        xpool = ctx.enter_context(tc.tile_pool(name="x", bufs=3))
        gpool = ctx.enter_context(tc.tile_pool(name="g", bufs=2))
        spool = ctx.enter_context(tc.tile_pool(name="s", bufs=4))
        opool = ctx.enter_context(tc.tile_pool(name="o", bufs=6))
        pspool = ctx.enter_context(tc.tile_pool(name="ps", bufs=8, space="PSUM"))

        # SBUF weight tiles: [P, 2*DK, FP] so [:, 2dk:2dk+2, mslice] is a
        # [128, 2, 128] DoubleRow stationary operand (256-deep contraction)
        w1h_sb = wpool.tile([P, 2 * DK, FP], e4, tag="w1h", name="w1h_sb")
        w1l_sb = wpool.tile([P, 2 * DK, FP], e4, tag="w1l", name="w1l_sb")
        w2h_sb = wpool.tile([P, 2 * DK, FP], e4, tag="w2h", name="w2h_sb")
        w2l_sb = wpool.tile([P, 2 * DK, FP], e4, tag="w2l", name="w2l_sb")
        w3h_sb = wpool.tile([P, 2 * FK, D], e4, tag="w3h", name="w3h_sb")
        w3l_sb = wpool.tile([P, 2 * FK, D], e4, tag="w3l", name="w3l_sb")

        # ---- DMA preload; first-matmul deps (w1h dk0 + x0h dk0) land first
        nn0 = chunks[0][1]
        x0h = xpool.tile([P, 2 * DK, nn0], e4, tag="xh", name="x0h")
        x0l = xpool.tile([P, 2 * DK, nn0], e4, tag="xl", name="x0l")
        loads = [
            (w1h_sb[:, 0:2], w1h[0:2].rearrange("k p f -> p k f")),
            (x0h[:, 0:2], xh[0:2, :, 0:nn0].rearrange("k p n -> p k n")),
            (w1h_sb[:, 2:4], w1h[2:4].rearrange("k p f -> p k f")),
            (x0h[:, 2:4], xh[2:4, :, 0:nn0].rearrange("k p n -> p k n")),
            (w1h_sb[:, 4:], w1h[4:].rearrange("k p f -> p k f")),
            (x0h[:, 4:], xh[4:, :, 0:nn0].rearrange("k p n -> p k n")),
            (x0l[:], xl[:, :, 0:nn0].rearrange("k p n -> p k n")),
            (w1l_sb[:], w1l.rearrange("k p f -> p k f")),
            (w2h_sb[:], w2h.rearrange("k p f -> p k f")),
            (w2l_sb[:], w2l.rearrange("k p f -> p k f")),
        ]
        wloads = [
            (w3h_sb[:], w3h.rearrange("k p d -> p k d")),
            (w3l_sb[:], w3l.rearrange("k p d -> p k d")),
        ]
        xtiles = {0: (x0h, x0l)}
        for ci, (n0, nn) in enumerate(chunks):
            if ci == 0:
                continue
            xnh = xpool.tile([P, 2 * DK, nn], e4, tag="xh", name=f"x{ci}h")
            xnl = xpool.tile([P, 2 * DK, nn], e4, tag="xl", name=f"x{ci}l")
            loads.append((xnh[:], xh[:, :, n0 : n0 + nn].rearrange("k p n -> p k n")))
            loads.append((xnl[:], xl[:, :, n0 : n0 + nn].rearrange("k p n -> p k n")))
            if ci == 1:
                loads.extend(wloads)  # w3 lands after x1 (needed later)
            xtiles[ci] = (xnh, xnl)
        # alternate the two HWDGE-capable queues (SP, ACT): the per-DMA
        # ~650ns SEQ+HWDGE issue cost would otherwise serialize the preload
        engines = (nc.sync, nc.scalar)
        for li, (dst, srcap) in enumerate(loads):
            engines[li % 2].dma_start(dst, srcap)

        def ms_slice(m):
            return slice(m * P, (m + 1) * P)

        gtiles = {}

        def _mm_group(ps, terms, ms):
            for ti, (w, xn) in enumerate(terms):
                for dk in range(DK):
                    for m in ms:
                        nc.tensor.matmul(
                            ps[m][:],
                            w[:, 2 * dk : 2 * dk + 2, ms_slice(m)],
                            xn[:, 2 * dk : 2 * dk + 2],
                            start=(ti == 0 and dk == 0),
                            stop=(ti == 2 and dk == DK - 1),
                            perf_mode=DR,
                        )

        def _chain(m, ps1, ps2, g_hi, g_lo, on_pool=False):
            sil = spool.tile([P, nn_of[m]], f32, tag="sil", name="sil")
            nc.scalar.activation(sil[:], ps1[m][:], silu, scale=C1)
            gsf = spool.tile([P, nn_of[m]], f32, tag="gsf", name="gsf")
            # gsf = (sil * CG) * ps2  == g * SG
            nc.vector.scalar_tensor_tensor(
                gsf[:], sil[:], CG, ps2[m][:],
                mybir.AluOpType.mult, mybir.AluOpType.mult,
            )
            if on_pool:
                # last chunk: quantize on Pool so ACT/DVE stay free for the
                # previous chunk's output copies
                nc.gpsimd.tensor_copy(g_hi[:, m], gsf[:])
                nc.gpsimd.tensor_sub(g_lo[:, m], gsf[:], g_hi[:, m])
            else:
                nc.scalar.activation(g_hi[:, m], gsf[:], copyf)
                nc.vector.tensor_sub(g_lo[:, m], gsf[:], g_hi[:, m])

        nn_of = {}

        def stage_a(ci):
            n0, nn = chunks[ci]
            xnh, xnl = xtiles[ci]
            for m in range(MT):
                nn_of[m] = nn
            # g tiles: [:, 2fk:2fk+2] is the [128, 2, nn] stage-B moving operand
            g_hi = gpool.tile([P, 2 * FK, nn], e4, tag="ghi", name=f"ghi{ci}")
            g_lo = gpool.tile([P, 2 * FK, nn], e4, tag="glo", name=f"glo{ci}")
            gtiles[ci] = (g_hi, g_lo)
            t1 = [(w1h_sb, xnh), (w1h_sb, xnl), (w1l_sb, xnh)]
            t2 = [(w2h_sb, xnh), (w2h_sb, xnl), (w2l_sb, xnh)]
            if ci == 0:
                # all 4 m-tiles for both passes (term-outer matches DMA
                # arrival order); chains run at the end, overlapped by A(1)
                ps1 = {
                    m: pspool.tile([P, nn], f32, tag="ps", name=f"ps1_{m}")
                    for m in range(MT)
                }
                _mm_group(ps1, t1, tuple(range(MT)))
                ps2 = {
                    m: pspool.tile([P, nn], f32, tag="ps", name=f"ps2_{m}")
                    for m in range(MT)
                }
                _mm_group(ps2, t2, tuple(range(MT)))
                for m in range(MT):
                    _chain(m, ps1, ps2, g_hi, g_lo)
                return
            for ms in ((0, 1), (2, 3)):
                ps1 = {
                    m: pspool.tile([P, nn], f32, tag="ps", name=f"ps1_{m}")
                    for m in ms
                }
                _mm_group(ps1, t1, ms)
                ps2 = {
                    m: pspool.tile([P, nn], f32, tag="ps", name=f"ps2_{m}")
                    for m in ms
                }
                _mm_group(ps2, t2, ms)
                for m in ms:
                    _chain(m, ps1, ps2, g_hi, g_lo)

        def stage_b(ci, is_last):
            n0, nn = chunks[ci]
            g_hi, g_lo = gtiles[ci]
            ot = None
            otfull = None
            if is_last:
                otfull = opool.tile([P, DT, nn], bf16, tag="otf", name="otfull")
            store_eng = (nc.scalar, nc.gpsimd, nc.sync)
            copy_eng = (nc.vector, nc.scalar) * 3
            for d in range(DT):
                pso = pspool.tile([P, nn], f32, tag="ps", name="pso")
                first, last = (0, 0), (1, 2)
                for fk in range(FK):
                    for ti, (w, g) in enumerate(
                        [(w3h_sb, g_hi), (w3l_sb, g_hi), (w3h_sb, g_lo)]
                    ):
                        nc.tensor.matmul(
                            pso[:],
                            w[:, 2 * fk : 2 * fk + 2, d * P : (d + 1) * P],
                            g[:, 2 * fk : 2 * fk + 2],
                            start=((fk, ti) == first),
                            stop=((fk, ti) == last),
                            perf_mode=DR,
                        )
                if is_last:
                    dst = otfull[:, d]
                else:
                    if d % 2 == 0:
                        ot = opool.tile(
                            [P, 2, nn], bf16, tag="ot", name=f"ot{d // 2}"
                        )
                    dst = ot[:, d % 2]
                ce = copy_eng[d]
                if ce is nc.scalar:
                    ce.activation(dst, pso[:], copyf, scale=CY)
                else:
                    ce.tensor_scalar_mul(dst, pso[:], CY)
                if not is_last and d % 2 == 1:
                    store_eng[d // 2].dma_start(youts[ci, d // 2][:], ot[:])
            if is_last:
                # one merged store: a single HWDGE slot in the tail window
                nc.sync.dma_start(youts[ci, 0][:], otfull[:])


        # software pipeline: A(i+1) is emitted before B(i) so the PE stream
        # never waits on chunk i's silu/quant chain
        nchunks = len(chunks)
        stage_a(0)
        for ci in range(1, nchunks):
            stage_a(ci)
            stage_b(ci - 1, False)
        stage_b(nchunks - 1, True)

    nc.compile()
    return nc


LAST_RESULTS = None  # BassKernelResults of the most recent run (for test harness)


def _e4(a):
    import ml_dtypes

    return np.ascontiguousarray(a).astype(ml_dtypes.float8_e4m3)


def _split_hi_lo(a32):
    """fp8 split: a32 (already scaled, fp32) -> (hi, lo) e4m3 at one scale."""
    hi = _e4(a32)
    lo = _e4(a32 - hi.astype(np.float32))
    return hi, lo


def _pack_dr(mat, kt):
    """[K, N] -> [2*kt, P, N] plain k-tile layout (DoubleRow pairs adjacent)."""
    K, N = mat.shape
    assert K == kt * 2 * P
    return np.ascontiguousarray(mat.reshape(2 * kt, P, N))


def kernel(x, Wg, bg, W1, W2, W3):
    global LAST_RESULTS
    from concourse.bass_utils import run_bass_kernel_spmd

    x = np.asarray(x)
    Wg, bg = np.asarray(Wg), np.asarray(bg)
    W1, W2, W3 = np.asarray(W1), np.asarray(W2), np.asarray(W3)
    B, S, d = x.shape
    T = B * S
    assert d == D and Wg.shape == (E, D)

    xf = np.ascontiguousarray(x.reshape(T, D))

    # ---- host gate + top-1 routing (fp64: exact vs any fp32 backend) ----
    gate = xf.astype(np.float64) @ Wg.astype(np.float64).T + bg.astype(np.float64)
    eid = np.argmax(gate, axis=1)
    counts = np.bincount(eid, minlength=E)
    order = np.argsort(eid, kind="stable")
    offs = np.concatenate(([0], np.cumsum(counts)))

    C = max(MIN_C, 2 * int(-(-counts.max() // 2)))
    if C not in _cache:
        _cache[C] = _build(C)
    nc = _cache[C]

    # ---- build per-core inputs (dispatch) ----
    in_maps = []
    tok_lists = []
    for e in range(E):
        toks = order[offs[e] : offs[e + 1]]
        tok_lists.append(toks)
        ce = len(toks)
        xeT = np.zeros((D, C), dtype=np.float32)
        if ce:
            xeT[:, :ce] = xf[toks].T
        xh_, xl_ = _split_hi_lo(xeT * SX)

        w1 = np.zeros((D, FP), dtype=np.float32)
        w1[:, :F] = W1[e].T
        w2 = np.zeros((D, FP), dtype=np.float32)
        w2[:, :F] = W2[e].T
        w3 = np.zeros((FP, D), dtype=np.float32)
        w3[:F, :] = W3[e].T
        w1h_, w1l_ = _split_hi_lo(w1 * SW)
        w2h_, w2l_ = _split_hi_lo(w2 * SW)
        w3h_, w3l_ = _split_hi_lo(w3 * SW)

        in_maps.append(
            {
                "xh": _pack_dr(xh_, DK),
                "xl": _pack_dr(xl_, DK),
                "w1h": _pack_dr(w1h_, DK),
                "w1l": _pack_dr(w1l_, DK),
                "w2h": _pack_dr(w2h_, DK),
                "w2l": _pack_dr(w2l_, DK),
                "w3h": _pack_dr(w3h_, FK),
                "w3l": _pack_dr(w3l_, FK),
            }
        )

    res = run_bass_kernel_spmd(nc, in_maps, list(range(E)))
    LAST_RESULTS = res

    # ---- combine: scatter outputs back to token order ----
    y = np.empty((T, D), dtype=np.float32)
    for e in range(E):
        toks = tok_lists[e]
        if len(toks):
            r = res.results[e]
            sizes = _chunks_of(C)
            cols = []
            for ci, nn in enumerate(sizes):
                if ci == len(sizes) - 1:
                    blk = r[f"y{ci}_0"].reshape(P, DT, nn).transpose(1, 0, 2)
                else:
                    pairs = [
                        r[f"y{ci}_{j}"].reshape(P, 2, nn)
                        for j in range(DT // 2)
                    ]
                    blk = (
                        np.stack(pairs, 0).transpose(0, 2, 1, 3)
                    )  # [3, 2, P, nn]
                cols.append(blk.reshape(D, nn))
            yte = np.concatenate(cols, axis=1).astype(np.float32)
            assert np.isfinite(yte).all(), "fp8 overflow on core %d" % e
            y[toks] = yte[:, : len(toks)].T
    return y.reshape(B, S, d)
